# revision 42
# baseline (speedup 1.0000x reference)
"""GCNEncoder (GCNConv + TransformerEncoderLayer) on 8 Trainium2 NeuronCores.

Sharding: nodes split 512/core. Per core:
  - GCN: dense normalized-adjacency blocks A [4096 src, 512 dst] built on
    device via GPSIMD local_scatter from host-prenormalized edge values
    (deg/dinv/dup-merge/self-loops folded in at host); aggregation runs
    TRANSPOSED (h^T = xw^T-stationary @ A) so h lands feature-major with no
    transposes anywhere in the pipeline.
  - Attention in fp8: q/K^T/V cast to fp8e4 (x16 scaled), ONE 2MB fp8
    AllGather of K/V, per-rank chunked gather loads so scores start early.
    exp probs kept fp8; PV and the softmax denominators use fp8 DoubleRow
    matmuls (2x, contraction pairs are the natural tile layout).
  - Post-attention stays transposed: o scaled by reciprocal-row broadcast,
    out_proj^T, LayerNorms via ones-matmul row reductions, FFN fp16 with
    ff2^T, final LN2^T written transposed; host un-transposes the output.
"""

import math

import numpy as np

import concourse.bacc as bacc
import concourse.mybir as mybir
import concourse.tile as tile
from concourse import library_config
from concourse.tile_rust import add_dep_helper

N_CORES = 8
N = 4096
E = 131072
DIN = 512
D = 256
H = 2
DH = 128
DFF = 2048
EPS = 1e-5
P = 128

NPC = N // N_CORES          # nodes per core = 512
MPC = NPC // P              # m-chunks per core = 4
KT = N // P                 # src k-tiles = 32
KPAD = 32                   # max out-edges per (core, src-node)
FSC = 16.0                  # fp8 pre-scale for q/k/v
DT8 = mybir.dt.float8e4
DT16 = mybir.dt.float16
DT32 = mybir.dt.float32
DTI16 = mybir.dt.int16
F = mybir.ActivationFunctionType
A = mybir.AluOpType
DR = mybir.MatmulPerfMode.DoubleRow
INV_SQRT_DH = 1.0 / math.sqrt(DH)
EXP_SCALE = INV_SQRT_DH / (FSC * FSC)


def build_kernel():
    nc = bacc.Bacc("TRN2", target_bir_lowering=False, debug=False,
                   num_devices=N_CORES)

    def din(name, shape, dt=DT32):
        return nc.dram_tensor(name, shape, dt, kind="ExternalInput")

    warr_d = din("warr", [P, KT * KPAD], DT16)
    idx_d = din("idx", [P, KT * KPAD], DTI16)
    xTf_d = din("xTf", [P, (DIN // P) * N], DT16)   # full x.T wrapped
    wg_d = din("wg", [P, (DIN // P) * D], DT16)
    winT_d = din("winT", [P, 2 * 3 * D], DT16)
    ipb_d = din("ipb", [P, 6])
    woT2_d = din("woT2", [P, 4 * P], DT16)
    w1T_d = din("w1T", [P, 2 * DFF], DT16)
    b1_d = din("b1", [P, DFF // P])
    w2T2_d = din("w2T2", [P, (DFF // P) * D], DT16)
    cols_d = din("cols", [P, 14])
    ident_d = din("ident", [P, P], DT16)

    out_d = nc.dram_tensor("out", [P, 2 * NPC], DT32, kind="ExternalOutput")

    with tile.TileContext(nc) as tc:
        with (
            tc.tile_pool(name="keep", bufs=1) as keep,
            tc.tile_pool(name="dram", bufs=1, space="DRAM") as dram,
        ):
            ones8 = keep.tile([P, 32], DT8)
            ones16c = keep.tile([P, 1], DT16)
            ones16r = keep.tile([1, P], DT16)
            eps128 = keep.tile([P, 1], DT32)
            nc.vector.memset(ones8[:], 1.0)
            nc.vector.memset(ones16c[:], 1.0)
            nc.vector.memset(ones16r[:], 1.0)
            nc.vector.memset(eps128[:], EPS)
            ident16 = keep.tile([P, P], DT16)
            nc.sync.dma_start(ident16[:], ident_d[:])
            ident1 = keep.tile([1, 1], DT32)
            nc.vector.memset(ident1[:], 1.0)

            lib = nc.gpsimd.load_library(library_config.local_scatter)

            gk = ctx_gcn = tc.tile_pool(name="gcn_keep", bufs=1)
            gk = ctx_gcn.__enter__()

            # ---- A-build inputs first: scatters on GpSimd start ASAP ----
            warr = gk.tile([P, KT * KPAD], DT16)
            idx_t = gk.tile([P, KT * KPAD], DTI16)
            nc.sync.dma_start(warr[:], warr_d[:])
            nc.sync.dma_start(idx_t[:], idx_d[:])

            a_tiles = [gk.tile([P, NPC], DT16, tag=f"A{kt}", name=f"A{kt}")
                       for kt in range(KT)]
            last_scatter = None
            for kt in range(KT):
                ls = nc.gpsimd.local_scatter(
                    a_tiles[kt][:],
                    warr[:, KPAD * kt:KPAD * (kt + 1)],
                    idx_t[:, KPAD * kt:KPAD * (kt + 1)],
                    channels=P, num_elems=NPC, num_idxs=KPAD,
                )
                add_dep_helper(ls.ins, lib.ins, reason="scatter after lib")
                last_scatter = ls

            # tiny warm-up AllGather after the scatters: absorbs the ~13us
            # ncfw cold-start so the real K gather begins immediately
            warm_bounce = dram.tile([P, 16], DT8)
            warm_gath = dram.tile([N_CORES * P, 16], DT8, addr_space="Shared")
            nc.scalar.dma_start(warm_bounce[:], ones8[:, 0:16])
            warm = nc.gpsimd.collective_compute(
                "AllGather", A.bypass,
                replica_groups=[list(range(N_CORES))],
                ins=[warm_bounce.opt()], outs=[warm_gath.opt()])
            add_dep_helper(warm.ins, last_scatter.ins,
                           reason="warm-up after scatters")

            xTf16 = gk.tile([P, (DIN // P) * N], DT16)
            wg16 = gk.tile([P, (DIN // P) * D], DT16)
            nc.sync.dma_start(wg16[:], wg_d[:])
            # chunk the 4MB x.T load by node range so the xw matmuls start
            # as soon as the first node stripe lands (all 4 k-stripes of it)
            NQ = 1024
            for q in range(N // NQ):
                for k in range(DIN // P):
                    nc.sync.dma_start(
                        xTf16[:, N * k + NQ * q:N * k + NQ * (q + 1)],
                        xTf_d[:, N * k + NQ * q:N * k + NQ * (q + 1)])

            cols = keep.tile([P, 14], DT32)
            winT16 = keep.tile([P, 2 * 3 * D], DT16)
            ipb = keep.tile([P, 6], DT32)
            nc.sync.dma_start(cols[:], cols_d[:])
            nc.sync.dma_start(winT16[:], winT_d[:])
            nc.sync.dma_start(ipb[:], ipb_d[:])

            # ---- xw = x @ W_gcn (replicated, fp16) ----
            xws16f = gk.tile([P, KT * D], DT16)
            with tc.tile_pool(name="xw_ps", bufs=4, space="PSUM") as xps:
                for j in range(KT):
                    pxw = xps.tile([P, D], DT32, space="PSUM", tag="xw")
                    for k in range(DIN // P):
                        nc.tensor.matmul(
                            pxw[:],
                            lhsT=xTf16[:, N * k + P * j:N * k + P * (j + 1)],
                            rhs=wg16[:, D * k:D * (k + 1)],
                            start=(k == 0), stop=(k == DIN // P - 1))
                    nc.vector.tensor_copy(xws16f[:, D * j:D * (j + 1)], pxw[:])

            # ---- transposed aggregation: h^T[c*128+p, dst] ----
            hT16 = keep.tile([P, 2 * NPC], DT16)
            with tc.tile_pool(name="agg_ps", bufs=1, space="PSUM") as aps:
                hps = [aps.tile([P, NPC], DT32, space="PSUM",
                                tag=f"hps{c}", name=f"hps{c}")
                       for c in range(2)]
                for kt in range(KT):
                    for c in range(2):
                        mm = nc.tensor.matmul(
                            hps[c][:],
                            lhsT=xws16f[:, D * kt + P * c:D * kt + P * (c + 1)],
                            rhs=a_tiles[kt][:],
                            start=(kt == 0), stop=(kt == KT - 1))
                        if kt == 0 and c == 0:
                            add_dep_helper(mm.ins, last_scatter.ins,
                                           reason="agg after scatters")
                for c in range(2):
                    nc.scalar.activation(hT16[:, NPC * c:NPC * (c + 1)],
                                         hps[c][:], F.Relu,
                                         bias=cols[:, c:c + 1])

            ctx_gcn.__exit__(None, None, None)
            ak = ctx_attn = tc.tile_pool(name="attn_keep", bufs=1)
            ak = ctx_attn.__enter__()

            # ---- local K^T / V in fp8 (x16), two pipelined AllGathers: K
            #      first (scores start as soon as it lands), V gathers in
            #      the background; q computed while the collectives run ----
            qT8 = ak.tile([P, H * NPC], DT8)
            k8_sb = ak.tile([P, 2 * NPC], DT8)
            v8_sb = ak.tile([P, 2 * NPC], DT8)
            with tc.tile_pool(name="kv_ps", bufs=3, space="PSUM") as kvps:
                for hh in range(H):
                    pk = kvps.tile([P, NPC], DT32, space="PSUM", tag="kv")
                    for k in range(2):
                        nc.tensor.matmul(
                            pk[:],
                            lhsT=winT16[:, 768 * k + D + P * hh:
                                        768 * k + D + P * (hh + 1)],
                            rhs=hT16[:, NPC * k:NPC * (k + 1)],
                            start=(k == 0), stop=(k == 1))
                    nc.vector.tensor_scalar(
                        k8_sb[:, NPC * hh:NPC * (hh + 1)], pk[:],
                        ipb[:, 2 + hh:3 + hh], FSC, op0=A.add, op1=A.mult)

                k_bounce = dram.tile([P, 2 * NPC], DT8)
                k_gath = dram.tile([N_CORES * P, 2 * NPC], DT8,
                                   addr_space="Shared")
                nc.scalar.dma_start(k_bounce[:], k8_sb[:])
                nc.gpsimd.collective_compute(
                    "AllGather", A.bypass,
                    replica_groups=[list(range(N_CORES))],
                    ins=[k_bounce.opt()], outs=[k_gath.opt()])

                for hh in range(H):
                    for m in range(MPC):
                        pv = kvps.tile([P, P], DT32, space="PSUM", tag="kvv")
                        for k in range(2):
                            nc.tensor.matmul(
                                pv[:],
                                lhsT=hT16[:, NPC * k + P * m:NPC * k + P * (m + 1)],
                                rhs=winT16[:, 768 * k + 2 * D + P * hh:
                                            768 * k + 2 * D + P * (hh + 1)],
                                start=(k == 0), stop=(k == 1))
                        nc.vector.tensor_scalar(
                            v8_sb[:, NPC * hh + P * m:
                                  NPC * hh + P * (m + 1)], pv[:],
                            FSC, None, op0=A.mult)

                v_bounce = dram.tile([P, 2 * NPC], DT8)
                v_gath = dram.tile([N_CORES * P, 2 * NPC], DT8,
                                   addr_space="Shared")
                nc.scalar.dma_start(v_bounce[:], v8_sb[:])
                nc.gpsimd.collective_compute(
                    "AllGather", A.bypass,
                    replica_groups=[list(range(N_CORES))],
                    ins=[v_bounce.opt()], outs=[v_gath.opt()])

                for hh in range(H):
                    pq = kvps.tile([P, NPC], DT32, space="PSUM", tag="kv")
                    for k in range(2):
                        nc.tensor.matmul(
                            pq[:],
                            lhsT=winT16[:, 768 * k + P * hh:768 * k + P * (hh + 1)],
                            rhs=hT16[:, NPC * k:NPC * (k + 1)],
                            start=(k == 0), stop=(k == 1))
                    nc.vector.tensor_scalar(
                        qT8[:, NPC * hh:NPC * (hh + 1)], pq[:],
                        ipb[:, hh:hh + 1], FSC, op0=A.add, op1=A.mult)

            # FFN / out-proj weights stream while the AllGather runs
            w1T16 = ak.tile([P, 2 * DFF], DT16)
            nc.sync.dma_start(w1T16[:], w1T_d[:])
            w2T216 = ak.tile([P, (DFF // P) * D], DT16)
            nc.sync.dma_start(w2T216[:], w2T2_d[:])
            woT216 = ak.tile([P, 4 * P], DT16)
            nc.sync.dma_start(woT216[:], woT2_d[:])
            b1t = ak.tile([P, DFF // P], DT32)
            nc.sync.dma_start(b1t[:], b1_d[:])

            # ---- per-rank loads of gathered K (V loads after its gather) ----
            k_all = ak.tile([P, N_CORES * 2 * NPC], DT8)
            v_all = ak.tile([P, N_CORES * 2 * NPC], DT8)
            for g in range(N_CORES):
                nc.scalar.dma_start(
                    k_all[:, 2 * NPC * g:2 * NPC * (g + 1)],
                    k_gath[P * g:P * (g + 1), :])
            for g in range(N_CORES):
                nc.scalar.dma_start(
                    v_all[:, 2 * NPC * g:2 * NPC * (g + 1)],
                    v_gath[P * g:P * (g + 1), :])

            def kslice(hh, kt):
                g, ktl = kt // MPC, kt % MPC
                base = 2 * NPC * g + NPC * hh + P * ktl
                return k_all[:, base:base + P]

            def vpair(hh, kt2):
                g, ml = kt2 // 2, kt2 % 2
                base = 2 * NPC * g + NPC * hh + 2 * P * ml
                return v_all[:, base:base + 2 * P].rearrange(
                    "p (two f) -> p two f", two=2)

            # ---- scores-first: S^T -> exp(fp8) + denominators while the V
            #      AllGather finishes; PV DoubleRow burst afterwards ----
            oS16 = ak.tile([P, H * NPC], DT16)
            es_all = ak.tile([P, 2 * KT * NPC], DT8)
            with tc.tile_pool(name="att_sb", bufs=3) as atsb, \
                 tc.tile_pool(name="att_ps", bufs=1, space="PSUM") as atps:
                o_ps = [atps.tile([P, NPC], DT32, space="PSUM",
                                  tag=f"o{hh}", name=f"o{hh}")
                        for hh in range(H)]
                sum_ps = [atps.tile([1, NPC], DT32, space="PSUM",
                                    tag=f"sm{hh}", name=f"sm{hh}")
                          for hh in range(H)]

                def esl(hh, kt2):
                    sl = es_all[:, 2 * NPC * (2 * kt2 + hh):
                                2 * NPC * (2 * kt2 + hh + 1)]
                    return sl.rearrange("p (two n) -> p two n", two=2)

                with tc.tile_pool(name="s_ps", bufs=2, space="PSUM") as sps:
                    for kt2 in range(KT // 2):
                        for hh in range(H):
                            ps_s = sps.tile([P, 2 * NPC], DT32, space="PSUM",
                                            tag="S")
                            for u in range(2):
                                kt = 2 * kt2 + u
                                nc.tensor.matmul(
                                    ps_s[:, NPC * u:NPC * (u + 1)],
                                    lhsT=kslice(hh, kt),
                                    rhs=qT8[:, NPC * hh:NPC * (hh + 1)],
                                    start=True, stop=True)
                            es = es_all[:, 2 * NPC * (2 * kt2 + hh):
                                        2 * NPC * (2 * kt2 + hh + 1)]
                            nc.scalar.activation(es, ps_s[:], F.Exp,
                                                 scale=EXP_SCALE)
                            nc.tensor.matmul(
                                sum_ps[hh][:],
                                lhsT=ones8[:].rearrange(
                                    "p (two f) -> p two f", two=2)[:, :, 0:1],
                                rhs=esl(hh, kt2), perf_mode=DR,
                                start=(kt2 == 0), stop=(kt2 == KT // 2 - 1))
                for kt2 in range(KT // 2):
                    for hh in range(H):
                        nc.tensor.matmul(
                            o_ps[hh][:],
                            lhsT=vpair(hh, kt2),
                            rhs=esl(hh, kt2), perf_mode=DR,
                            start=(kt2 == 0), stop=(kt2 == KT // 2 - 1))

                # denominators: pack rows into partitions (reciprocal free
                # size is the cost driver), recip [128,8], broadcast back
                # via identity matmuls
                with tc.tile_pool(name="rb_ps", bufs=2, space="PSUM") as rps:
                    srow32 = atsb.tile([1, H * NPC], DT32, tag="srow")
                    for hh in range(H):
                        nc.vector.tensor_copy(
                            srow32[:, NPC * hh:NPC * (hh + 1)], sum_ps[hh][:])
                    packT = rps.tile([P, 2 * MPC], DT32, space="PSUM",
                                     tag="packT")
                    for j in range(2 * MPC):
                        nc.tensor.transpose(
                            packT[:, j:j + 1], srow32[:, P * j:P * (j + 1)],
                            ident1[:])
                    recT16 = atsb.tile([P, 2 * MPC], DT16, tag="recT")
                    with nc.allow_low_precision(
                            reason="softmax denom ~4096, f16 rel ok"):
                        nc.vector.reciprocal(recT16[:], packT[:])
                    for hh in range(H):
                        rbc = rps.tile([P, NPC], DT32, space="PSUM", tag="rbc")
                        for m in range(MPC):
                            nc.tensor.matmul(
                                rbc[:, P * m:P * (m + 1)],
                                lhsT=recT16[:, MPC * hh + m:
                                            MPC * hh + m + 1].to_broadcast(
                                    [P, P]),
                                rhs=ident16[:], start=True, stop=True)
                        rb16 = atsb.tile([P, NPC], DT16, tag="rb16")
                        nc.vector.tensor_copy(rb16[:], rbc[:])
                        nc.vector.tensor_tensor(
                            oS16[:, NPC * hh:NPC * (hh + 1)],
                            o_ps[hh][:], rb16[:], op=A.mult)

            # ---- out_proj^T + residual + LN1^T (all feature-major) ----
            h1T16 = ak.tile([P, 2 * NPC], DT16)
            x1h = ak.tile([P, 2 * NPC], DT16)
            with tc.tile_pool(name="ln_sb", bufs=2) as lsb:
                with tc.tile_pool(name="op_ps", bufs=1, space="PSUM") as ops:
                    x1_ps = [ops.tile([P, NPC], DT32, space="PSUM",
                                      tag=f"x1{c}", name=f"x1{c}")
                             for c in range(2)]
                    for c in range(2):
                        for hh in range(H):
                            nc.tensor.matmul(
                                x1_ps[c][:],
                                lhsT=woT216[:, P * (2 * hh + c):
                                            P * (2 * hh + c + 1)],
                                rhs=oS16[:, NPC * hh:NPC * (hh + 1)],
                                start=(hh == 0), stop=(hh == 1))
                    for c in range(2):
                        nc.vector.scalar_tensor_tensor(
                            x1h[:, NPC * c:NPC * (c + 1)], x1_ps[c][:],
                            cols[:, 2 + c:3 + c],
                            hT16[:, NPC * c:NPC * (c + 1)],
                            op0=A.add, op1=A.add)

                def layernorm_T(dst, xh, gcol, bcol, out_dt, tag):
                    """LN over features (partition dim x 2 chunks), rows via
                    ones-matmuls. xh: [P, 2*NPC] f16. dst written per chunk."""
                    with tc.tile_pool(name=f"ln_ps_{tag}", bufs=1,
                                      space="PSUM") as rws:
                        mu_ps = rws.tile([1, NPC], DT32, space="PSUM",
                                         tag=f"{tag}mu")
                        msq_ps = rws.tile([1, NPC], DT32, space="PSUM",
                                          tag=f"{tag}ms")
                        sq = lsb.tile([P, 2 * NPC], DT16, tag=f"{tag}sq")
                        nc.vector.tensor_tensor(sq[:], xh[:], xh[:], op=A.mult)
                        for c in range(2):
                            nc.tensor.matmul(
                                mu_ps[:], lhsT=ones16c[:],
                                rhs=xh[:, NPC * c:NPC * (c + 1)],
                                start=(c == 0), stop=(c == 1))
                        for c in range(2):
                            nc.tensor.matmul(
                                msq_ps[:], lhsT=ones16c[:],
                                rhs=sq[:, NPC * c:NPC * (c + 1)],
                                start=(c == 0), stop=(c == 1))
                        mu_n = lsb.tile([1, NPC], DT32, tag=f"{tag}mn")
                        nc.vector.tensor_scalar(mu_n[:], mu_ps[:], 1.0 / D,
                                                None, op0=A.mult)
                        nmu16 = lsb.tile([1, NPC], DT16, tag=f"{tag}nm")
                        nc.vector.tensor_scalar(nmu16[:], mu_ps[:], -1.0 / D,
                                                None, op0=A.mult)
                        mu2 = lsb.tile([1, NPC], DT32, tag=f"{tag}m2")
                        nc.vector.tensor_tensor(mu2[:], mu_n[:], mu_n[:],
                                                op=A.mult)
                        var32 = lsb.tile([1, NPC], DT32, tag=f"{tag}vr")
                        nc.vector.scalar_tensor_tensor(
                            var32[:], msq_ps[:], 1.0 / D, mu2[:],
                            op0=A.mult, op1=A.subtract)
                        # rstd via partition-packed sqrt+recip (cheap free dim)
                        packV = rws.tile([P, MPC], DT32, space="PSUM",
                                         tag=f"{tag}pk")
                        for m in range(MPC):
                            nc.tensor.transpose(
                                packV[:, m:m + 1],
                                var32[:, P * m:P * (m + 1)],
                                ident1[:])
                        sdT = lsb.tile([P, MPC], DT32, tag=f"{tag}sd")
                        nc.scalar.activation(sdT[:], packV[:], F.Sqrt,
                                             bias=eps128[:])
                        rstdT16 = lsb.tile([P, MPC], DT16, tag=f"{tag}rs")
                        with nc.allow_low_precision(
                                reason="rstd f16, rel 1e-3 ok"):
                            nc.vector.reciprocal(rstdT16[:], sdT[:])
                        nmu_bc = rws.tile([P, NPC], DT32, space="PSUM",
                                          tag=f"{tag}nb")
                        rstd_bc = rws.tile([P, NPC], DT32, space="PSUM",
                                           tag=f"{tag}rb")
                        nc.tensor.matmul(nmu_bc[:], lhsT=ones16r[:],
                                         rhs=nmu16[:], start=True, stop=True)
                        for m in range(MPC):
                            nc.tensor.matmul(
                                rstd_bc[:, P * m:P * (m + 1)],
                                lhsT=rstdT16[:, m:m + 1].to_broadcast([P, P]),
                                rhs=ident16[:], start=True, stop=True)
                        for c in range(2):
                            t = lsb.tile([P, NPC], DT16, tag=f"{tag}t")
                            nc.vector.tensor_tensor(
                                t[:], xh[:, NPC * c:NPC * (c + 1)], nmu_bc[:],
                                op=A.add)
                            t2 = lsb.tile([P, NPC], DT16, tag=f"{tag}t2")
                            nc.vector.tensor_tensor(t2[:], t[:], rstd_bc[:],
                                                    op=A.mult)
                            nc.vector.tensor_scalar(
                                dst[:, NPC * c:NPC * (c + 1)], t2[:],
                                gcol[:, c:c + 1], bcol[:, c:c + 1],
                                op0=A.mult, op1=A.add)

                layernorm_T(h1T16, x1h, cols[:, 6:8], cols[:, 8:10], DT16, "a")

                # ---- FFN (fp16, transposed ff2, ff2 interleaved one dc
                #      behind ff1 so relu pipelines under the matmuls) ----
                ff1T = ak.tile([P, (DFF // P) * NPC], DT16)
                x2h = lsb.tile([P, 2 * NPC], DT16, tag="x2h")
                NDC = DFF // P
                with tc.tile_pool(name="f1_ps", bufs=3, space="PSUM") as fps, \
                     tc.tile_pool(name="f2_ps", bufs=1, space="PSUM") as fps2:
                    x2_ps = [fps2.tile([P, NPC], DT32, space="PSUM",
                                       tag=f"x2{c}", name=f"x2{c}")
                             for c in range(2)]

                    def ff2_step(dc):
                        for c in range(2):
                            nc.tensor.matmul(
                                x2_ps[c][:],
                                lhsT=w2T216[:, P * (2 * dc + c):
                                            P * (2 * dc + c + 1)],
                                rhs=ff1T[:, NPC * dc:NPC * (dc + 1)],
                                start=(dc == 0), stop=(dc == NDC - 1))

                    for dc in range(NDC):
                        pf = fps.tile([P, NPC], DT32, space="PSUM", tag="f1")
                        for k in range(2):
                            nc.tensor.matmul(
                                pf[:],
                                lhsT=w1T16[:, DFF * k + P * dc:
                                           DFF * k + P * (dc + 1)],
                                rhs=h1T16[:, NPC * k:NPC * (k + 1)],
                                start=(k == 0), stop=(k == 1))
                        nc.scalar.activation(
                            ff1T[:, NPC * dc:NPC * (dc + 1)], pf[:], F.Relu,
                            bias=b1t[:, dc:dc + 1])
                        if dc >= 1:
                            ff2_step(dc - 1)
                    ff2_step(NDC - 1)
                    for c in range(2):
                        nc.vector.scalar_tensor_tensor(
                            x2h[:, NPC * c:NPC * (c + 1)], x2_ps[c][:],
                            cols[:, 4 + c:5 + c],
                            h1T16[:, NPC * c:NPC * (c + 1)],
                            op0=A.add, op1=A.add)

                out_sb = ak.tile([P, 2 * NPC], DT32)
                layernorm_T(out_sb, x2h, cols[:, 10:12], cols[:, 12:14],
                            DT32, "b")
                nc.scalar.dma_start(out_d[:], out_sb[:])
            ctx_attn.__exit__(None, None, None)

    nc.compile()
    return nc


# ======================= host-side prep =======================

def _prep_inputs(x, edge_index, edge_weight, W_gcn, b_gcn, in_proj_w,
                 in_proj_b, out_proj_w, out_proj_b, lin1_w, lin1_b, lin2_w,
                 lin2_b, ln1_g, ln1_b, ln2_g, ln2_b):
    """Index-permutation / layout prep + edge-weight prenormalization."""
    x = np.asarray(x, np.float32)
    src = np.asarray(edge_index[0], np.int64)
    dst = np.asarray(edge_index[1], np.int64)
    w = np.asarray(edge_weight, np.float64)

    def wrap128(a):
        n = a.shape[0] // P
        return np.ascontiguousarray(
            a.reshape(n, P, a.shape[1]).transpose(1, 0, 2).reshape(P, -1))

    def colsof(v):
        return np.ascontiguousarray(
            np.asarray(v, np.float32).reshape(2, P).T)

    f16 = np.float16
    deg = np.zeros(N, np.float64)
    np.add.at(deg, dst, w)
    deg += 1.0
    dinv = 1.0 / np.sqrt(deg)
    norm = (dinv[src] * w * dinv[dst]).astype(np.float32)

    ipb_np = np.asarray(in_proj_b, np.float32)
    bv = ipb_np[2 * D:]
    bo_eff = (np.asarray(out_proj_w, np.float32) @ bv
              + np.asarray(out_proj_b, np.float32))

    wo = np.asarray(out_proj_w, np.float32)
    woT2 = np.empty((P, 4 * P), np.float32)
    for hh in range(H):
        for c in range(2):
            # lhsT[p, m] = Wo[c*128+m, hh*128+p] / FSC
            woT2[:, P * (2 * hh + c):P * (2 * hh + c + 1)] = \
                wo[c * P:(c + 1) * P, hh * P:(hh + 1) * P].T / FSC

    w2 = np.asarray(lin2_w, np.float32)
    w2T2 = np.empty((P, (DFF // P) * D), np.float32)
    for dc in range(DFF // P):
        for c in range(2):
            w2T2[:, P * (2 * dc + c):P * (2 * dc + c + 1)] = \
                w2[c * P:(c + 1) * P, dc * P:(dc + 1) * P].T

    cols = np.concatenate([
        colsof(b_gcn), colsof(bo_eff), colsof(lin2_b),
        colsof(ln1_g), colsof(ln1_b), colsof(ln2_g), colsof(ln2_b)], axis=1)

    shared = {
        "xTf": wrap128(np.ascontiguousarray(x.T)).astype(f16),
        "wg": wrap128(np.asarray(W_gcn, np.float32)).astype(f16),
        "winT": wrap128(np.ascontiguousarray(
            np.asarray(in_proj_w, np.float32).T)).astype(f16),
        "ipb": np.ascontiguousarray(ipb_np.reshape(6, P).T),
        "woT2": woT2.astype(f16),
        "w1T": wrap128(np.ascontiguousarray(
            np.asarray(lin1_w, np.float32).T)).astype(f16),
        "b1": np.ascontiguousarray(
            np.asarray(lin1_b, np.float32).reshape(DFF // P, P).T),
        "w2T2": w2T2.astype(f16),
        "cols": cols,
        "ident": np.eye(P, dtype=f16),
    }

    core_of = dst // NPC
    in_maps = []
    for c in range(N_CORES):
        sel = np.nonzero(core_of == c)[0]
        s_c = src[sel]
        d_c = (dst[sel] - NPC * c).astype(np.int64)
        n_c = norm[sel]

        w_arr = np.zeros((N, KPAD), np.float32)
        idx_arr = np.full((N, KPAD), -1, np.int16)
        counts = np.zeros(N, np.int32)
        slot_of = {}
        for si, di, wi in zip(s_c.tolist(), d_c.tolist(), n_c.tolist()):
            key = si * NPC + di
            slot = slot_of.get(key)
            if slot is None:
                j = int(counts[si])
                assert j < KPAD, f"KPAD overflow at src {si}"
                counts[si] = j + 1
                w_arr[si, j] = wi
                idx_arr[si, j] = di
                slot_of[key] = j
            else:
                w_arr[si, slot] += wi
        # self loops: weight dinv^2 at (g, g-512c)
        for di in range(NPC):
            g = NPC * c + di
            key = g * NPC + di
            slot = slot_of.get(key)
            if slot is None:
                j = int(counts[g])
                assert j < KPAD, f"KPAD overflow at self {g}"
                counts[g] = j + 1
                w_arr[g, j] = dinv[g] * dinv[g]
                idx_arr[g, j] = di
            else:
                w_arr[g, slot] += dinv[g] * dinv[g]

        in_maps.append({
            **shared,
            "warr": wrap128(w_arr).astype(f16),
            "idx": wrap128(idx_arr),
        })
    return in_maps


# ======================= runner =======================

class _Runner:
    """Persistent-jit SPMD executor (mirrors bass2jax.run_bass_via_pjrt)."""

    def __init__(self, nc):
        import jax
        from jax.sharding import Mesh, PartitionSpec
        from jax.experimental.shard_map import shard_map
        from concourse.bass2jax import (_bass_exec_p, install_neuronx_cc_hook,
                                        partition_id_tensor)
        install_neuronx_cc_hook()
        self.jax = jax
        partition_name = (nc.partition_id_tensor.name
                          if nc.partition_id_tensor else None)
        in_names, out_names, out_avals, zero_outs = [], [], [], []
        for alloc in nc.m.functions[0].allocations:
            if not isinstance(alloc, mybir.MemoryLocationSet):
                continue
            name = alloc.memorylocations[0].name
            if alloc.kind == "ExternalInput":
                if name != partition_name:
                    in_names.append(name)
            elif alloc.kind == "ExternalOutput":
                out_names.append(name)
                shape = tuple(alloc.tensor_shape)
                dtype = mybir.dt.np(alloc.dtype)
                out_avals.append(jax.core.ShapedArray(shape, dtype))
                zero_outs.append(np.zeros(shape, dtype))
        self.in_names, self.out_names = in_names, out_names
        self.out_shapes = [tuple(a.shape) for a in out_avals]
        self.n_params = len(in_names)
        self.zero_outs = zero_outs
        all_in = in_names + out_names
        if partition_name is not None:
            all_in.append(partition_name)

        def _body(*args):
            operands = list(args)
            if partition_name is not None:
                operands.append(partition_id_tensor())
            return tuple(_bass_exec_p.bind(
                *operands, out_avals=tuple(out_avals), in_names=tuple(all_in),
                out_names=tuple(out_names), lowering_input_output_aliases=(),
                sim_require_finite=True, sim_require_nnan=True, nc=nc))

        devices = jax.devices()[:N_CORES]
        self.mesh = Mesh(np.asarray(devices), ("core",))
        nin = self.n_params + len(out_names)
        self.fn = jax.jit(
            shard_map(_body, mesh=self.mesh,
                      in_specs=(PartitionSpec("core"),) * nin,
                      out_specs=(PartitionSpec("core"),) * len(out_names),
                      check_rep=False),
            keep_unused=True)

    def place(self, in_maps):
        import jax
        from jax.sharding import PartitionSpec
        per_core = [[np.asarray(m[n]) for n in self.in_names] for m in in_maps]
        concat = [np.concatenate([per_core[c][i] for c in range(N_CORES)], axis=0)
                  for i in range(self.n_params)]
        zeros = [np.zeros((N_CORES * z.shape[0], *z.shape[1:]), z.dtype)
                 for z in self.zero_outs]
        sh = jax.sharding.NamedSharding(self.mesh, PartitionSpec("core"))
        return [jax.device_put(a, sh) for a in (*concat, *zeros)]

    def run(self, args):
        outs = self.fn(*args)
        self.jax.block_until_ready(outs)
        return outs

    def results(self, outs):
        res = []
        for c in range(N_CORES):
            d = {}
            for i, name in enumerate(self.out_names):
                full = np.asarray(outs[i])
                ps = self.out_shapes[i]
                d[name] = full.reshape((N_CORES,) + ps)[c]
            res.append(d)
        return res


_CACHE = {}


def _get_runner():
    if "runner" not in _CACHE:
        nc = build_kernel()
        _CACHE["nc"] = nc
        _CACHE["runner"] = _Runner(nc)
    return _CACHE["runner"]


def kernel(**inputs) -> np.ndarray:
    runner = _get_runner()
    in_maps = _prep_inputs(**inputs)
    args = runner.place(in_maps)
    outs = runner.run(args)
    res = runner.results(outs)
    # out_d is out^T wrapped: [P, 2, NPC]; out[n, c*128+p] = arr[p, c, n]
    full = np.empty((N, D), np.float32)
    for c in range(N_CORES):
        arr = res[c]["out"].reshape(P, 2, NPC)
        full[NPC * c:NPC * (c + 1)] = arr.transpose(2, 1, 0).reshape(NPC, D)
    return full
